# revision 5
# baseline (speedup 1.0000x reference)
"""Trainium2 Bass kernel for the GNN pair-scoring decoder loss.

Math (see reference):
    For each pair row (i0, i1):
        a = [e0[i0], e1[i0]]  (128 floats),  b = [e0[i1], e1[i1]]
        x = concat_{i,j in {0,1}} a[64i:64i+64] * b[64j:64j+64]   # [256]
        scr = exp(sigmoid(relu(relu(x@W1+b1)@W2+b2)@W3+b3))
    Per batch row: 1 pos pair + 32 neg pairs; denom = pos + sum(negs)
    loss = -sum_b (pos_b / (denom_b + 1e-8) + 1e-8)

Strategy: pure data parallel over the 16384 batch rows; 8 cores x 2048
rows.  Each core processes 2048*33 = 67584 pair rows:
  - host packs a [200000, 128] bf16 embedding table (both layers per row)
    and per-core index arrays laid out so a batch row's 33 scores end up
    contiguous in the free dim of one partition.
  - device gathers 2x 256B vectors per pair row via indirect DMA,
    builds the 256-wide pair feature, transposes through the PE, and runs
    the 3-layer MLP with bf16 matmuls (f32 PSUM accumulate).
  - scores staged in SBUF [128, 528]; segmented (33-wide) free-dim
    reduction gives denominators; the per-partition partial sums of
    pos/denom go back to the host, which finishes the scalar loss.
"""

import contextlib
import sys

import numpy as np

try:
    import concourse  # noqa: F401
except ImportError:  # pragma: no cover
    sys.path.insert(0, "/opt/trn_rl_repo")

import ml_dtypes
import concourse.bass as bass
import concourse.bacc as bacc
import concourse.mybir as mybir
import concourse.tile as tile
from concourse.masks import make_identity

F32 = mybir.dt.float32
BF16 = mybir.dt.bfloat16
I32 = mybir.dt.int32
AF = mybir.ActivationFunctionType

# Problem constants (hardcoded per the harness contract)
L, N_NODES, D = 2, 200000, 64
B, K = 16384, 32
N_CORES = 8
B_CORE = B // N_CORES            # 2048 batch rows per core
SLOTS = K + 1                    # 33 scores per batch row (pos first)
N_COLS = B_CORE * SLOTS // 128   # 528 chunk columns of 128 pair rows
GB = 48                          # chunk columns gathered per indirect DMA
NG = N_COLS // GB                # 11 gather batches
TPG = GB // 4                    # 12 mm-tiles (4 chunks = 512 rows) per batch

_PROGRAM = None


def _build_program(loop_r=None):
    """loop_r: if set, wrap the computation in a For_i loop repeating it
    loop_r times (identical, idempotent passes) — benchmarking only."""
    nc = bacc.Bacc("TRN2", target_bir_lowering=False, debug=False)

    embT = nc.dram_tensor("embT", [N_NODES, 2 * D], BF16, kind="ExternalInput")
    i0_d = nc.dram_tensor("i0", [128, N_COLS], I32, kind="ExternalInput")
    i1_d = nc.dram_tensor("i1", [128, N_COLS], I32, kind="ExternalInput")
    w1_d = nc.dram_tensor("w1", [256, 128], BF16, kind="ExternalInput")
    w2_d = nc.dram_tensor("w2", [128, 64], BF16, kind="ExternalInput")
    w3_d = nc.dram_tensor("w3", [64, 1], BF16, kind="ExternalInput")
    b1_d = nc.dram_tensor("b1", [128, 1], F32, kind="ExternalInput")
    b2_d = nc.dram_tensor("b2", [64, 1], F32, kind="ExternalInput")
    b3_d = nc.dram_tensor("b3", [128, 1], F32, kind="ExternalInput")
    out_d = nc.dram_tensor("out_part", [128, 1], F32, kind="ExternalOutput")

    with tile.TileContext(nc) as tc:
        with (
            tc.tile_pool(name="const", bufs=1) as cp,
            tc.tile_pool(name="gather", bufs=2) as gp,
            tc.tile_pool(name="work", bufs=2) as wp,
            tc.tile_pool(name="psum", bufs=2, space="PSUM") as pp,
        ):
            i0_sb = cp.tile([128, N_COLS], I32, tag="i0")
            i1_sb = cp.tile([128, N_COLS], I32, tag="i1")
            w1a = cp.tile([128, 128], BF16, tag="w1a")
            w1b = cp.tile([128, 128], BF16, tag="w1b")
            w2s = cp.tile([128, 64], BF16, tag="w2")
            w3s = cp.tile([64, 1], BF16, tag="w3")
            b1s = cp.tile([128, 1], F32, tag="b1")
            b2s = cp.tile([64, 1], F32, tag="b2")
            b3s = cp.tile([128, 1], F32, tag="b3")
            ident = cp.tile([128, 128], BF16, tag="ident")
            stage = cp.tile([128, N_COLS], F32, tag="stage")

            nc.sync.dma_start(out=i0_sb[:], in_=i0_d[:])
            nc.sync.dma_start(out=i1_sb[:], in_=i1_d[:])
            nc.sync.dma_start(out=w1a[:], in_=w1_d[0:128, :])
            nc.sync.dma_start(out=w1b[:], in_=w1_d[128:256, :])
            nc.sync.dma_start(out=w2s[:], in_=w2_d[:])
            nc.sync.dma_start(out=w3s[:], in_=w3_d[:])
            nc.sync.dma_start(out=b1s[:], in_=b1_d[:])
            nc.sync.dma_start(out=b2s[:], in_=b2_d[:])
            nc.sync.dma_start(out=b3s[:], in_=b3_d[:])
            make_identity(nc, ident[:])

            def emit_body():
                for g in range(NG):
                    A = gp.tile([128, GB * 128], BF16, tag="A")
                    Bt = gp.tile([128, GB * 128], BF16, tag="B")
                    c0 = g * GB
                    nc.gpsimd.indirect_dma_start(
                        out=A[:], out_offset=None, in_=embT[:],
                        in_offset=bass.IndirectOffsetOnAxis(
                            ap=i0_sb[:, c0:c0 + GB], axis=0),
                    )
                    nc.gpsimd.indirect_dma_start(
                        out=Bt[:], out_offset=None, in_=embT[:],
                        in_offset=bass.IndirectOffsetOnAxis(
                            ap=i1_sb[:, c0:c0 + GB], axis=0),
                    )

                    for t in range(TPG):
                        X = wp.tile([128, 1024], BF16, tag="X")
                        for c in range(4):
                            cg = t * 4 + c
                            a128 = A[:, cg * 128:(cg + 1) * 128]
                            b128 = Bt[:, cg * 128:(cg + 1) * 128]
                            in0 = (a128.rearrange("p (i k) -> p i k", i=2)
                                   .unsqueeze(2).to_broadcast([128, 2, 2, 64]))
                            in1 = (b128.rearrange("p (j k) -> p j k", j=2)
                                   .unsqueeze(1).to_broadcast([128, 2, 2, 64]))
                            outx = X[:, c * 256:(c + 1) * 256].rearrange(
                                "p (i j k) -> p i j k", i=2, j=2)
                            nc.vector.tensor_tensor(
                                out=outx, in0=in0, in1=in1,
                                op=mybir.AluOpType.mult)

                        # transpose the 4x [128, 256] chunks into X^T
                        # halves, packed in one bf16 PSUM bank:
                        # cols [0,512) = feats 0..127, [512,1024) = 128..255
                        ptk = pp.tile([128, 1024], BF16, tag="ptk")
                        for c in range(4):
                            nc.tensor.transpose(
                                out=ptk[:, c * 128:(c + 1) * 128],
                                in_=X[:, c * 256:c * 256 + 128],
                                identity=ident[:])
                            nc.tensor.transpose(
                                out=ptk[:, 512 + c * 128:512 + (c + 1) * 128],
                                in_=X[:, c * 256 + 128:c * 256 + 256],
                                identity=ident[:])
                        XT = wp.tile([128, 1024], BF16, tag="XT")
                        nc.vector.tensor_copy(out=XT[:], in_=ptk[:])

                        z1 = pp.tile([128, 512], F32, tag="z1")
                        nc.tensor.matmul(out=z1[:], lhsT=w1a[:],
                                         rhs=XT[:, 0:512],
                                         start=True, stop=False)
                        nc.tensor.matmul(out=z1[:], lhsT=w1b[:],
                                         rhs=XT[:, 512:1024],
                                         start=False, stop=True)
                        h1 = wp.tile([128, 512], BF16, tag="h1")
                        nc.scalar.activation(out=h1[:], in_=z1[:],
                                             func=AF.Relu, bias=b1s[:])

                        z2 = pp.tile([64, 512], F32, tag="z2")
                        nc.tensor.matmul(out=z2[:], lhsT=w2s[:], rhs=h1[:],
                                         start=True, stop=True)
                        h2 = wp.tile([64, 512], BF16, tag="h2")
                        nc.scalar.activation(out=h2[:], in_=z2[:],
                                             func=AF.Relu, bias=b2s[:])

                        z3 = pp.tile([128, 4], F32, tag="z3")
                        for c in range(4):
                            nc.tensor.matmul(out=z3[:, c:c + 1],
                                             lhsT=h2[:, c * 128:(c + 1) * 128],
                                             rhs=w3s[:],
                                             start=True, stop=True)
                        T = g * TPG + t
                        nc.scalar.activation(out=stage[:, 4 * T:4 * T + 4],
                                             in_=z3[:], func=AF.Sigmoid,
                                             bias=b3s[:])

                # ---- final: scores -> per-partition partial sums ----
                acts = cp.tile([128, N_COLS], F32, tag="acts")
                nc.scalar.activation(out=acts[:], in_=stage[:], func=AF.Exp)
                den = cp.tile([128, 16], F32, tag="den")
                nc.vector.reduce_sum(
                    out=den[:],
                    in_=acts[:].rearrange("p (g s) -> p g s", s=SLOTS),
                    axis=mybir.AxisListType.X)
                den2 = cp.tile([128, 16], F32, tag="den2")
                nc.vector.tensor_scalar_add(out=den2[:], in0=den[:],
                                            scalar1=1e-8)
                rec = cp.tile([128, 16], F32, tag="rec")
                nc.vector.reciprocal(out=rec[:], in_=den2[:])
                ratio = cp.tile([128, 16], F32, tag="ratio")
                posv = acts[:].rearrange("p (g s) -> p g s", s=SLOTS)[:, :, 0]
                nc.vector.tensor_tensor(out=ratio[:], in0=posv, in1=rec[:],
                                        op=mybir.AluOpType.mult)
                part = cp.tile([128, 1], F32, tag="part")
                nc.vector.reduce_sum(out=part[:], in_=ratio[:],
                                     axis=mybir.AxisListType.X)
                nc.sync.dma_start(out=out_d[:], in_=part[:])

            loop_cm = (tc.For_i(0, loop_r, 1) if loop_r
                       else contextlib.nullcontext())
            with loop_cm:
                emit_body()

    nc.compile()
    return nc


def get_program():
    global _PROGRAM
    if _PROGRAM is None:
        _PROGRAM = _build_program()
    return _PROGRAM


def _prep_inputs(embeds, pos, neg, W1, b1, W2, b2, W3, b3):
    """Host-side packing: returns (shared dict, [per-core dicts])."""
    bf16 = ml_dtypes.bfloat16
    embT = np.concatenate([embeds[0], embeds[1]], axis=1).astype(bf16)
    shared = {
        "embT": embT,
        "w1": np.asarray(W1, np.float32).astype(bf16),
        "w2": np.asarray(W2, np.float32).astype(bf16),
        "w3": np.asarray(W3, np.float32).astype(bf16),
        "b1": np.asarray(b1, np.float32).reshape(128, 1),
        "b2": np.asarray(b2, np.float32).reshape(64, 1),
        "b3": np.full((128, 1), np.float32(np.asarray(b3).reshape(-1)[0])),
    }
    pos = np.asarray(pos)
    neg = np.asarray(neg)
    per_core = []
    for m in range(N_CORES):
        sl = slice(m * B_CORE, (m + 1) * B_CORE)
        I0 = np.empty((B_CORE, SLOTS), np.int32)
        I1 = np.empty((B_CORE, SLOTS), np.int32)
        I0[:, 0] = pos[sl, 0]
        I1[:, 0] = pos[sl, 1]
        I0[:, 1:] = neg[sl, :, 0]
        I1[:, 1:] = neg[sl, :, 1]
        per_core.append({
            "i0": np.ascontiguousarray(I0.reshape(128, N_COLS)),
            "i1": np.ascontiguousarray(I1.reshape(128, N_COLS)),
        })
    return shared, per_core


def _finish(partials):
    """partials: list of [128,1] f32 arrays -> scalar loss."""
    total = np.float64(0.0)
    for p in partials:
        total += np.asarray(p, np.float64).sum()
    loss = -(np.float32(total) + np.float32(B * 1e-8))
    return np.asarray(loss, np.float32)


def kernel(**inputs):
    from concourse.bass_utils import run_bass_kernel_spmd

    nc = get_program()
    shared, per_core = _prep_inputs(**inputs)
    in_maps = [{**shared, **pc} for pc in per_core]
    res = run_bass_kernel_spmd(nc, in_maps, core_ids=list(range(N_CORES)))
    partials = [r["out_part"] for r in res.results]
    return _finish(partials)


# ---------------------------------------------------------------------------
# numpy emulation of the device math (for sim/host validation)
def _host_emulate_core(shared, pc):
    bf16 = ml_dtypes.bfloat16
    embT = np.asarray(shared["embT"], np.float32)
    i0 = pc["i0"].reshape(-1)
    i1 = pc["i1"].reshape(-1)  # [p, col] flat order
    a = embT[i0]
    b = embT[i1]
    x = np.empty((a.shape[0], 256), np.float32)
    for i in range(2):
        for j in range(2):
            x[:, (2 * i + j) * 64:(2 * i + j + 1) * 64] = (
                a[:, i * 64:(i + 1) * 64] * b[:, j * 64:(j + 1) * 64])
    x = x.astype(bf16).astype(np.float32)
    w1 = np.asarray(shared["w1"], np.float32)
    w2 = np.asarray(shared["w2"], np.float32)
    w3 = np.asarray(shared["w3"], np.float32)
    z1 = x @ w1 + shared["b1"].reshape(-1)
    h1 = np.maximum(z1, 0).astype(bf16).astype(np.float32)
    z2 = h1 @ w2 + shared["b2"].reshape(-1)
    h2 = np.maximum(z2, 0).astype(bf16).astype(np.float32)
    z3 = (h2 @ w3).reshape(-1) + shared["b3"][0, 0]
    s = 1.0 / (1.0 + np.exp(-z3))
    e = np.exp(s)
    e = e.reshape(128, 16, SLOTS)
    den = e.sum(-1) + 1e-8
    return (e[:, :, 0] / den).sum(-1, keepdims=True).astype(np.float32)


def _sim_check():
    from concourse.bass_interp import CoreSim
    rng = np.random.default_rng(0)
    embeds = rng.standard_normal((L, N_NODES, D), np.float32)
    pos = rng.integers(0, N_NODES, (B, 2)).astype(np.int32)
    neg = rng.integers(0, N_NODES, (B, K, 2)).astype(np.int32)
    din = 256
    W1 = rng.uniform(-0.1, 0.1, (din, 128)).astype(np.float32)
    W2 = rng.uniform(-0.1, 0.1, (128, 64)).astype(np.float32)
    W3 = rng.uniform(-0.3, 0.3, (64, 1)).astype(np.float32)
    b1 = np.zeros(128, np.float32)
    b2 = np.zeros(64, np.float32)
    b3 = np.zeros(1, np.float32)
    shared, per_core = _prep_inputs(embeds, pos, neg, W1, b1, W2, b2, W3, b3)

    nc = get_program()
    sim = CoreSim(nc, trace=False)
    for k_, v in {**shared, **per_core[0]}.items():
        sim.tensor(k_)[:] = v
    print("simulating ...")
    sim.simulate(check_with_hw=False)
    got = np.array(sim.tensor("out_part"))
    want = _host_emulate_core(shared, per_core[0])
    err = np.abs(got - want) / (np.abs(want) + 1e-6)
    print("sim partial[:4]", got[:4, 0], "want", want[:4, 0])
    print("max rel err vs bf16-emu:", err.max())
    print("modeled exec time:", sim.time, "ns")
    assert err.max() < 2e-2, err.max()
    print("SIM CHECK PASSED")


if __name__ == "__main__":
    if len(sys.argv) > 1 and sys.argv[1] == "sim":
        _sim_check()
